# revision 47
# baseline (speedup 1.0000x reference)
"""BagModel (segment_reduce) Trainium2 kernel.

Computes out = (1/64 * segment_sum(relu(x @ W1 + b1))) @ W2 + b2 for
4096 bags of exactly 64 consecutive rows each, sharded bag-aligned
across 8 NeuronCores (512 bags / 32768 rows per core, weights
replicated, no cross-core communication).

Layout trick: the host permutes each core's x-shard to
    xh[p, g, k*512 + b] = x[b*64 + g, 128*k + p]
so row-group g contains row g of every bag, with the contraction dim D
on partitions.  The per-bag segment-sum then falls out of PSUM matmul
accumulation: the second (W2) matmul accumulates over the 64 row-groups
with start=(g==0)/stop=(g==63), so no explicit reduction pass over h is
ever needed.  The 4 H-slices of the W2 matmul go to 4 distinct PE
column-groups (tile_position) so they overlap in the array.

fp8: x and W1 are quantized to e4m3 on the host and the W1 matmul runs
in DoubleRow perf mode (both 128-row contraction halves in a single
double-pumped pass), halving PE time on the dominant matmul.  htr/W2
stay bf16 (fp8 there would push rel-err past the 2e-2 gate).
"""

import numpy as np

import concourse.bass as bass
import concourse.tile as tile
from concourse import bacc, mybir
from concourse.tile import add_dep_helper

N, D, H, C = 262144, 256, 512, 10
N_BAGS, BAG_SIZE = 4096, 64
N_CORES = 8
R = N // N_CORES            # rows per core
BPC = N_BAGS // N_CORES     # bags per core == free dim of each row-group
KT = D // 128               # contraction tiles (2)
MT = H // 128               # H tiles (4)

F32 = mybir.dt.float32
BF16 = mybir.dt.bfloat16
FP8 = mybir.dt.float8e4
AF = mybir.ActivationFunctionType
ALU = mybir.AluOpType

# compute dtypes: x/W1 in fp8e4m3 + DoubleRow double-pumping (2x PE rate
# on the dominant matmul); htr/W2 in bf16 (accumulation stays fp32 in
# PSUM either way)
CDT = BF16

# relu tile split: scalar engine (~590ns/tile) vs vector (~671ns/tile);
# balanced at ~137/119 of the 256 tiles: scalar takes m<2 plus m==2 for
# the first SCALAR_M2 groups
SCALAR_M2 = 9


def build(nc: bass.Bass, bag: int = BAG_SIZE, bpc: int = BPC):
    """Emit the per-core program.  bag = rows per bag (= number of
    row-groups), bpc = bags per core (= free dim, <= 512)."""
    xT = nc.declare_dram_parameter("xh", [128, bag, KT, bpc], FP8,
                                   isOutput=False)
    w1 = nc.declare_dram_parameter("w1h", [128, KT, H], FP8, isOutput=False)
    b1 = nc.declare_dram_parameter("b1h", [128, MT], F32, isOutput=False)
    w2 = nc.declare_dram_parameter("w2h", [128, MT, C], CDT, isOutput=False)
    b2 = nc.declare_dram_parameter("b2h", [C, 1], F32, isOutput=False)
    out = nc.declare_dram_parameter("out", [C, bpc], F32, isOutput=True)

    with tile.TileContext(nc) as tc:
        with (
            tc.tile_pool(name="const", bufs=1) as cpool,
            # bufs=8 matches the 8-queue HWDGE rotation: slot reuse then
            # pairs WAW deps on the same queue (implicit FIFO, no extra
            # sync wait — walrus allows only one non-self wait per inst)
            tc.tile_pool(name="xin", bufs=8) as xpool,
            tc.tile_pool(name="hrelu", bufs=10) as hpool,
            tc.tile_pool(name="fin", bufs=1) as fpool,
            tc.tile_pool(name="w2c", bufs=2) as w2cpool,
            tc.tile_pool(name="ps_ht", bufs=7, space="PSUM") as pspool,
            tc.tile_pool(name="ps_out", bufs=1, space="PSUM") as popool,
        ):
            # const loads go on the Activation HWDGE queue so the SP queue
            # starts issuing the (latency-critical) first x tiles at once;
            # w1 split per m-slice across queues so the first main matmul
            # gates on a 256B/partition transfer only
            w1_sb = cpool.tile([128, KT, H], FP8)
            for m in range(MT):
                nc.scalar.dma_start(
                    out=w1_sb[:, :, 128 * m:128 * (m + 1)],
                    in_=w1[:, :, 128 * m:128 * (m + 1)],
                )
            b1_sb = cpool.tile([128, MT], F32)
            nc.scalar.dma_start(out=b1_sb[:], in_=b1[:])
            w2_sb = cpool.tile([128, MT, C], CDT)
            nc.scalar.dma_start(out=w2_sb[:], in_=w2[:])
            b2_sb = cpool.tile([C, 1], F32)
            nc.scalar.dma_start(out=b2_sb[:], in_=b2[:])

            # bag-sum accumulators: col-group m holds partial (over H slice
            # m) of out.T at partitions [32m, 32m+10).  The has_written
            # clear of start=True is per-partition (HW-verified), so all
            # four col-groups share ONE bank on disjoint partitions —
            # freeing a 7th bank for ht pipelining (deeper W1 runway).
            out_bank = popool.tile([128, bpc], F32, tag="outb",
                                   name="out_psb")
            out_ps = [out_bank for _ in range(MT)]

            # W2 matmuls are batched into chunks of CH groups, gated by
            # an explicit dependency: a tiny gpsimd copy of the W2
            # weights into a per-chunk tile depends (add_dep_helper) on
            # ALL the chunk's relu instructions, and the chunk's quads
            # read that copy as lhsT.  The whole 4*CH-matmul burst thus
            # becomes ready at one instant and runs as one same-mode
            # block (its emission priority beats any later W1).  Each
            # DoubleRow<->normal transition costs ~100-150ns (cross-mode
            # LDWEIGHTS can't prefetch into the array); a
            # readiness-driven interleaved schedule pays that 4x per
            # group (~32us) — chunked bursts pay it 2x per chunk (~4us).
            # The 4 W2 matmuls of a group go to 4 distinct PE column
            # groups and overlap in the array.
            CH = 4              # groups per W2 chunk

            def emit_w2(gprev, htr_prev, w2_src):
                for m in range(MT):
                    nc.tensor.matmul(
                        out_ps[m][32 * m:32 * m + C, :],
                        lhsT=w2_src[:, m, :],
                        rhs=htr_prev[m][:],
                        start=(gprev == 0),
                        stop=(gprev == bag - 1),
                        tile_position=(0, 32 * m),
                        skip_group_check=True,
                    )

            pending = []      # (g, htrs) awaiting the chunk-end W2 flush
            chunk_relus = []  # relu instructions of the current chunk
            for g in range(bag):
                # one fp8 tile [128, KT, bpc] (1KB/partition) per group;
                # the very first group is quarter-split so its transfers
                # parallelize across queues (cuts head latency).
                xt = xpool.tile([128, KT, bpc], FP8, tag="xt",
                                name=f"x_{g}")
                if g == 0:
                    quart = bpc // 4
                    for k in range(KT):
                        for q in range(2):
                            nc.sync.dma_start(
                                out=xt[:, k, 2 * q * quart:2 * (q + 1) * quart],
                                in_=xT[:, g, k, 2 * q * quart:2 * (q + 1) * quart],
                            )
                else:
                    nc.sync.dma_start(out=xt[:], in_=xT[:, g])
                htrs = []
                for m in range(MT):
                    ht = pspool.tile([128, bpc], F32, tag="ht")
                    if g == 0:
                        # free-dim halves matching the split DMAs: the
                        # first sub-matmul gates on one 128KB transfer
                        # through contended start-up HBM.
                        half = bpc // 2
                        for q in range(2):
                            nc.tensor.matmul(
                                ht[:, q * half:(q + 1) * half],
                                lhsT=w1_sb[:, :, 128 * m:128 * (m + 1)],
                                rhs=xt[:, :, q * half:(q + 1) * half],
                                start=True,
                                stop=True,
                                perf_mode=mybir.MatmulPerfMode.DoubleRow,
                            )
                    else:
                        nc.tensor.matmul(
                            ht[:],
                            lhsT=w1_sb[:, :, 128 * m:128 * (m + 1)],
                            rhs=xt[:],
                            start=True,
                            stop=True,
                            perf_mode=mybir.MatmulPerfMode.DoubleRow,
                        )
                    htr = hpool.tile([128, bpc], CDT, tag=f"htr{m}",
                                     name=f"htr_{g}_{m}")
                    on_scalar = m < 2 or (m == 2 and g < SCALAR_M2)
                    if on_scalar:
                        rb = nc.scalar.activation(
                            htr[:], ht[:], AF.Relu,
                            bias=b1_sb[:, m:m + 1], scale=1.0,
                        )
                    else:
                        rb = nc.vector.tensor_scalar(
                            out=htr[:], in0=ht[:],
                            scalar1=b1_sb[:, m:m + 1], scalar2=0.0,
                            op0=ALU.add, op1=ALU.max,
                        )
                    chunk_relus.append(rb)
                    htrs.append(htr)
                pending.append((g, htrs))
                # the final chunk is split so only one group's quads
                # trail the very last relu (shorter tail)
                if (g + 1) % CH == 0 or g >= bag - 2:
                    w2_c = w2cpool.tile([128, MT, C], CDT, tag="w2c",
                                        name=f"w2c_{g}")
                    cp = nc.gpsimd.tensor_copy(out=w2_c[:], in_=w2_sb[:])
                    for rb in chunk_relus:
                        add_dep_helper(cp.ins, rb.ins, sync=True,
                                       reason="w2 chunk gate")
                    for gp, h in pending:
                        emit_w2(gp, h, w2_c)
                    pending = []
                    chunk_relus = []

            # combine the 4 partials + b2 (each op may read only one PSUM
            # operand; b2 fused into the first op)
            acc = fpool.tile([C, bpc], F32, tag="acc")
            nc.vector.tensor_scalar(
                out=acc[:], in0=out_ps[0][0:C, :], scalar1=b2_sb[:],
                scalar2=None, op0=ALU.add,
            )
            for m in range(1, MT):
                nc.vector.tensor_add(
                    acc[:], acc[:], out_ps[m][32 * m:32 * m + C, :])
            nc.sync.dma_start(out=out[:], in_=acc[:])


def _np_fp8():
    import ml_dtypes
    return ml_dtypes.float8_e4m3


def _np_bf16():
    import ml_dtypes
    return ml_dtypes.bfloat16


def host_prep_shared(W1, b1, W2, b2, bag=BAG_SIZE):
    w1h = np.ascontiguousarray(
        W1.reshape(KT, 128, H).transpose(1, 0, 2)).astype(_np_fp8())
    b1h = np.ascontiguousarray(
        b1.reshape(MT, 128).T).astype(np.float32)
    w2h = np.ascontiguousarray(
        (W2 / bag).reshape(MT, 128, C).transpose(1, 0, 2)).astype(_np_bf16())
    b2h = np.ascontiguousarray(b2.reshape(C, 1)).astype(np.float32)
    return {"w1h": w1h, "b1h": b1h, "w2h": w2h, "b2h": b2h}


def host_prep_x(xs, bag=BAG_SIZE):
    """xs: [r, D] rows of one core -> xh [128, bag, KT, bpc] permuted
    so each row-group g is one contiguous 1KB/partition chunk."""
    r = xs.shape[0]
    bpc = r // bag
    xh = xs.reshape(bpc, bag, KT, 128).transpose(3, 1, 2, 0)
    return np.ascontiguousarray(xh).astype(_np_fp8())


_BUILT = None


def _get_built():
    global _BUILT
    if _BUILT is None:
        nc = bacc.Bacc("TRN2")
        build(nc)
        nc.compile()
        _BUILT = nc
    return _BUILT


def run(x, W1, b1, W2, b2, ids=None, trace=False):
    from concourse.bass_utils import run_bass_kernel_spmd

    nc = _get_built()
    shared = host_prep_shared(W1, b1, W2, b2)
    in_maps = []
    for c in range(N_CORES):
        xs = np.asarray(x[c * R:(c + 1) * R])
        in_maps.append({"xh": host_prep_x(xs), **shared})
    res = run_bass_kernel_spmd(
        nc, in_maps, core_ids=list(range(N_CORES)), trace=trace
    )
    outs = [res.results[c]["out"] for c in range(N_CORES)]
    full = np.concatenate([o.T for o in outs], axis=0).astype(np.float32)
    return full, res


def kernel(x, W1, b1, W2, b2, ids=None):
    full, _ = run(x, W1, b1, W2, b2, ids)
    return full


# revision 48
# speedup vs baseline: 1.2012x; 1.2012x over previous
"""BagModel (segment_reduce) Trainium2 kernel.

Computes out = (1/64 * segment_sum(relu(x @ W1 + b1))) @ W2 + b2 for
4096 bags of exactly 64 consecutive rows each, sharded bag-aligned
across 8 NeuronCores (512 bags / 32768 rows per core, weights
replicated, no cross-core communication).

Layout trick: the host permutes each core's x-shard to
    xh[p, g, k*512 + b] = x[b*64 + g, 128*k + p]
so row-group g contains row g of every bag, with the contraction dim D
on partitions.  The per-bag segment-sum then falls out of PSUM matmul
accumulation: the second (W2) matmul accumulates over the 64 row-groups
with start=(g==0)/stop=(g==63), so no explicit reduction pass over h is
ever needed.  The 4 H-slices of the W2 matmul go to 4 distinct PE
column-groups (tile_position) so they overlap in the array.

fp8: x and W1 are quantized to e4m3 on the host and the W1 matmul runs
in DoubleRow perf mode (both 128-row contraction halves in a single
double-pumped pass), halving PE time on the dominant matmul.  htr/W2
stay bf16 (fp8 there would push rel-err past the 2e-2 gate).
"""

import numpy as np

import concourse.bass as bass
import concourse.tile as tile
from concourse import bacc, mybir
from concourse.tile import add_dep_helper

N, D, H, C = 262144, 256, 512, 10
N_BAGS, BAG_SIZE = 4096, 64
N_CORES = 8
R = N // N_CORES            # rows per core
BPC = N_BAGS // N_CORES     # bags per core == free dim of each row-group
KT = D // 128               # contraction tiles (2)
MT = H // 128               # H tiles (4)

F32 = mybir.dt.float32
BF16 = mybir.dt.bfloat16
FP8 = mybir.dt.float8e4
AF = mybir.ActivationFunctionType
ALU = mybir.AluOpType

# compute dtypes: x/W1 in fp8e4m3 + DoubleRow double-pumping (2x PE rate
# on the dominant matmul); htr/W2 in bf16 (accumulation stays fp32 in
# PSUM either way)
CDT = BF16

# relu tile split: scalar engine (~633ns/tile) vs vector (~704ns/tile);
# balanced at ~135/121 of the 256 tiles: scalar takes m<2 plus m==2 for
# the first SCALAR_M2 groups
SCALAR_M2 = 7


def build(nc: bass.Bass, bag: int = BAG_SIZE, bpc: int = BPC):
    """Emit the per-core program.  bag = rows per bag (= number of
    row-groups), bpc = bags per core (= free dim, <= 512)."""
    xT = nc.declare_dram_parameter("xh", [128, bag, KT, bpc], FP8,
                                   isOutput=False)
    w1 = nc.declare_dram_parameter("w1h", [128, KT, H], FP8, isOutput=False)
    b1 = nc.declare_dram_parameter("b1h", [128, MT], F32, isOutput=False)
    w2 = nc.declare_dram_parameter("w2h", [128, MT, C], CDT, isOutput=False)
    b2 = nc.declare_dram_parameter("b2h", [C, 1], F32, isOutput=False)
    out = nc.declare_dram_parameter("out", [C, bpc], F32, isOutput=True)

    with tile.TileContext(nc) as tc:
        with (
            tc.tile_pool(name="const", bufs=1) as cpool,
            # bufs=8 matches the 8-queue HWDGE rotation: slot reuse then
            # pairs WAW deps on the same queue (implicit FIFO, no extra
            # sync wait — walrus allows only one non-self wait per inst)
            tc.tile_pool(name="xin", bufs=8) as xpool,
            tc.tile_pool(name="hrelu", bufs=10) as hpool,
            tc.tile_pool(name="fin", bufs=1) as fpool,
            tc.tile_pool(name="w2c", bufs=2) as w2cpool,
            tc.tile_pool(name="ps_ht", bufs=7, space="PSUM") as pspool,
            tc.tile_pool(name="ps_out", bufs=1, space="PSUM") as popool,
        ):
            # const loads go on the Activation HWDGE queue so the SP queue
            # starts issuing the (latency-critical) first x tiles at once;
            # w1 split per m-slice across queues so the first main matmul
            # gates on a 256B/partition transfer only
            w1_sb = cpool.tile([128, KT, H], FP8)
            for m in range(MT):
                nc.scalar.dma_start(
                    out=w1_sb[:, :, 128 * m:128 * (m + 1)],
                    in_=w1[:, :, 128 * m:128 * (m + 1)],
                )
            b1_sb = cpool.tile([128, MT], F32)
            nc.scalar.dma_start(out=b1_sb[:], in_=b1[:])
            w2_sb = cpool.tile([128, MT, C], CDT)
            nc.scalar.dma_start(out=w2_sb[:], in_=w2[:])
            b2_sb = cpool.tile([C, 1], F32)
            nc.scalar.dma_start(out=b2_sb[:], in_=b2[:])

            # bag-sum accumulators: col-group m holds partial (over H slice
            # m) of out.T at partitions [32m, 32m+10).  The has_written
            # clear of start=True is per-partition (HW-verified), so all
            # four col-groups share ONE bank on disjoint partitions —
            # freeing a 7th bank for ht pipelining (deeper W1 runway).
            out_bank = popool.tile([128, bpc], F32, tag="outb",
                                   name="out_psb")
            out_ps = [out_bank for _ in range(MT)]

            # W2 matmuls are batched into chunks of CH groups, gated by
            # an explicit dependency: a tiny gpsimd copy of the W2
            # weights into a per-chunk tile depends (add_dep_helper) on
            # ALL the chunk's relu instructions, and the chunk's quads
            # read that copy as lhsT.  The whole 4*CH-matmul burst thus
            # becomes ready at one instant and runs as one same-mode
            # block (its emission priority beats any later W1).  Each
            # DoubleRow<->normal transition costs ~100-150ns (cross-mode
            # LDWEIGHTS can't prefetch into the array); a
            # readiness-driven interleaved schedule pays that 4x per
            # group (~32us) — chunked bursts pay it 2x per chunk (~4us).
            # The 4 W2 matmuls of a group go to 4 distinct PE column
            # groups and overlap in the array.
            CH = 4              # groups per W2 chunk

            def emit_w2(gprev, htr_prev, w2_src):
                for m in range(MT):
                    nc.tensor.matmul(
                        out_ps[m][32 * m:32 * m + C, :],
                        lhsT=w2_src[:, m, :],
                        rhs=htr_prev[m][:],
                        start=(gprev == 0),
                        stop=(gprev == bag - 1),
                        tile_position=(0, 32 * m),
                        skip_group_check=True,
                    )

            pending = []      # (g, htrs) awaiting the chunk-end W2 flush
            chunk_relus = []  # relu instructions of the current chunk
            for g in range(bag):
                # one fp8 tile [128, KT, bpc] (1KB/partition) per group;
                # the very first group is quarter-split so its transfers
                # parallelize across queues (cuts head latency).
                xt = xpool.tile([128, KT, bpc], FP8, tag="xt",
                                name=f"x_{g}")
                if g == 0:
                    quart = bpc // 4
                    for k in range(KT):
                        for q in range(2):
                            nc.sync.dma_start(
                                out=xt[:, k, 2 * q * quart:2 * (q + 1) * quart],
                                in_=xT[:, g, k, 2 * q * quart:2 * (q + 1) * quart],
                            )
                else:
                    nc.sync.dma_start(out=xt[:], in_=xT[:, g])
                htrs = []
                for m in range(MT):
                    ht = pspool.tile([128, bpc], F32, tag="ht")
                    if g == 0:
                        # free-dim halves matching the split DMAs: the
                        # first sub-matmul gates on one 128KB transfer
                        # through contended start-up HBM.
                        half = bpc // 2
                        for q in range(2):
                            nc.tensor.matmul(
                                ht[:, q * half:(q + 1) * half],
                                lhsT=w1_sb[:, :, 128 * m:128 * (m + 1)],
                                rhs=xt[:, :, q * half:(q + 1) * half],
                                start=True,
                                stop=True,
                                perf_mode=mybir.MatmulPerfMode.DoubleRow,
                            )
                    else:
                        nc.tensor.matmul(
                            ht[:],
                            lhsT=w1_sb[:, :, 128 * m:128 * (m + 1)],
                            rhs=xt[:],
                            start=True,
                            stop=True,
                            perf_mode=mybir.MatmulPerfMode.DoubleRow,
                        )
                    htr = hpool.tile([128, bpc], CDT, tag=f"htr{m}",
                                     name=f"htr_{g}_{m}")
                    on_scalar = m < 2 or (m == 2 and g < SCALAR_M2)
                    if on_scalar:
                        rb = nc.scalar.activation(
                            htr[:], ht[:], AF.Relu,
                            bias=b1_sb[:, m:m + 1], scale=1.0,
                        )
                    else:
                        rb = nc.vector.tensor_scalar(
                            out=htr[:], in0=ht[:],
                            scalar1=b1_sb[:, m:m + 1], scalar2=0.0,
                            op0=ALU.add, op1=ALU.max,
                        )
                    chunk_relus.append(rb)
                    htrs.append(htr)
                pending.append((g, htrs))
                if (g + 1) % CH == 0 or g == bag - 1:
                    w2_c = w2cpool.tile([128, MT, C], CDT, tag="w2c",
                                        name=f"w2c_{g}")
                    cp = nc.gpsimd.tensor_copy(out=w2_c[:], in_=w2_sb[:])
                    for rb in chunk_relus:
                        add_dep_helper(cp.ins, rb.ins, sync=True,
                                       reason="w2 chunk gate")
                    for gp, h in pending:
                        emit_w2(gp, h, w2_c)
                    pending = []
                    chunk_relus = []

            # combine the 4 partials + b2 (each op may read only one PSUM
            # operand; b2 fused into the first op)
            acc = fpool.tile([C, bpc], F32, tag="acc")
            nc.vector.tensor_scalar(
                out=acc[:], in0=out_ps[0][0:C, :], scalar1=b2_sb[:],
                scalar2=None, op0=ALU.add,
            )
            for m in range(1, MT):
                nc.vector.tensor_add(
                    acc[:], acc[:], out_ps[m][32 * m:32 * m + C, :])
            nc.sync.dma_start(out=out[:], in_=acc[:])


def _np_fp8():
    import ml_dtypes
    return ml_dtypes.float8_e4m3


def _np_bf16():
    import ml_dtypes
    return ml_dtypes.bfloat16


def host_prep_shared(W1, b1, W2, b2, bag=BAG_SIZE):
    w1h = np.ascontiguousarray(
        W1.reshape(KT, 128, H).transpose(1, 0, 2)).astype(_np_fp8())
    b1h = np.ascontiguousarray(
        b1.reshape(MT, 128).T).astype(np.float32)
    w2h = np.ascontiguousarray(
        (W2 / bag).reshape(MT, 128, C).transpose(1, 0, 2)).astype(_np_bf16())
    b2h = np.ascontiguousarray(b2.reshape(C, 1)).astype(np.float32)
    return {"w1h": w1h, "b1h": b1h, "w2h": w2h, "b2h": b2h}


def host_prep_x(xs, bag=BAG_SIZE):
    """xs: [r, D] rows of one core -> xh [128, bag, KT, bpc] permuted
    so each row-group g is one contiguous 1KB/partition chunk."""
    r = xs.shape[0]
    bpc = r // bag
    xh = xs.reshape(bpc, bag, KT, 128).transpose(3, 1, 2, 0)
    return np.ascontiguousarray(xh).astype(_np_fp8())


_BUILT = None


def _get_built():
    global _BUILT
    if _BUILT is None:
        nc = bacc.Bacc("TRN2")
        build(nc)
        nc.compile()
        _BUILT = nc
    return _BUILT


def run(x, W1, b1, W2, b2, ids=None, trace=False):
    from concourse.bass_utils import run_bass_kernel_spmd

    nc = _get_built()
    shared = host_prep_shared(W1, b1, W2, b2)
    in_maps = []
    for c in range(N_CORES):
        xs = np.asarray(x[c * R:(c + 1) * R])
        in_maps.append({"xh": host_prep_x(xs), **shared})
    res = run_bass_kernel_spmd(
        nc, in_maps, core_ids=list(range(N_CORES)), trace=trace
    )
    outs = [res.results[c]["out"] for c in range(N_CORES)]
    full = np.concatenate([o.T for o in outs], axis=0).astype(np.float32)
    return full, res


def kernel(x, W1, b1, W2, b2, ids=None):
    full, _ = run(x, W1, b1, W2, b2, ids)
    return full
